# revision 38
# baseline (speedup 1.0000x reference)
"""Multi-head causal attention (B=2, S=2048, D=1024, H=16) on 8 trn2 cores.

Sharding: core c -> batch b = c//4, head-group g = c%4 (4 heads each).
Tensor-parallel on Wq/Wk/Wv (column) and Wo (row); the Wo all-reduce is the
host-side sum of the 4 per-core partials of each batch.

v2 changes over the fp32 baseline:
 - bf16 activations/weights end-to-end (host casts); PSUM stays fp32.
   Halves HBM traffic (37.6 -> ~19 MB/core) and SBUF footprint.
 - Causal diagonal trim: for diagonal-straddling key blocks only the
   query range >= the block's first key is scored/exp'd/accumulated.
   Cuts ACT exp work ~15% and trims score/AV streams.
 - Softmax sum reciprocal straight from PSUM on DVE (drops the ACT
   identity+eps hop; causal rows always have >= 1 unmasked key).
 - Batched DMA (2 issues per tensor-chunk, 1 output DMA per query
   superblock); chunk loads ride the sync queue so exp never queues
   behind DMA-issue on the scalar engine.
 - Fine-grained issue interleave: projection / output-projection
   micro-ops are woven between attention kb-steps so the in-order PE
   queue always has independent work adjacent (no head-of-line stall
   when exp rate-limits the attention inner loop).

Device layout notes (unchanged):
 - Activations stay transposed (features on partitions): every matmul
   contraction is on partitions with zero on-chip transposes.
 - Scores are built as S.T (keys on partitions, queries free); softmax
   sums come free via an appended ones-column of V.
 - No row-max subtraction: scores ~N(0, 0.4) after the 1/8 scale.
 - Causal mask applied post-exp, only on the [128,128] triangle of
   diagonal blocks.
"""

import numpy as np
import ml_dtypes

import concourse.bacc as bacc
import concourse.mybir as mybir
import concourse.tile as tile
from concourse.bass_utils import run_bass_kernel_spmd

B, S, D, H = 2, 2048, 1024, 16
DK = 64            # head dim
HG = 4             # heads per core
GD = HG * DK       # 256 dims per head-group
P = 128
NQ = 512           # query chunk (free dim of score blocks)
QB = S // NQ       # 4 query superblocks
KB = S // P        # 16 key blocks
KO = D // P        # 8 contraction tiles for the projections
F32 = mybir.dt.float32
BF16 = mybir.dt.bfloat16
BFNP = ml_dtypes.bfloat16
WARM_N = 10


def build(mode):
    assert mode in ("tril", "ones", "general")
    nc = bacc.Bacc(None, target_bir_lowering=False)

    # All DRAM layouts are partition-contiguous (host pre-arranges): every
    # DMA is 128 lines of >= 4KB, so HWDGE descriptor generation is cheap.
    xqT = nc.dram_tensor("xqT", [P, QB, KO, NQ], BF16, kind="ExternalInput")
    xkT = nc.dram_tensor("xkT", [P, QB, KO, NQ], BF16, kind="ExternalInput")
    xvT = nc.dram_tensor("xvT", [P, QB, KO, NQ], BF16, kind="ExternalInput")
    wqT = nc.dram_tensor("wqT", [P, KO, GD], BF16, kind="ExternalInput")
    wkT = nc.dram_tensor("wkT", [P, KO, GD], BF16, kind="ExternalInput")
    wvT = nc.dram_tensor("wvT", [P, KO, GD], BF16, kind="ExternalInput")
    woT = nc.dram_tensor("woT", [P, 2, D], BF16, kind="ExternalInput")
    maskd = maskT = None
    if mode == "tril":
        # [P, P] upper-triangular (key p <= query q') bf16 pattern
        maskd = nc.dram_tensor("maskd", [P, P], BF16, kind="ExternalInput")
    elif mode == "general":
        maskT = nc.dram_tensor("maskT", [S, S], BF16, kind="ExternalInput")
    outT = nc.dram_tensor("outT", [QB, P, KO, NQ], BF16, kind="ExternalOutput")

    with tile.TileContext(nc) as tc:
        with (
            tc.tile_pool(name="wpool", bufs=1) as wpool,
            tc.tile_pool(name="perm", bufs=1) as perm,
            tc.tile_pool(name="xs", bufs=6) as xsp,
            tc.tile_pool(name="es", bufs=6) as esp,
            tc.tile_pool(name="ob", bufs=2) as obp,
            tc.tile_pool(name="outp", bufs=2) as outp,
            tc.tile_pool(name="small", bufs=4) as smallp,
            tc.tile_pool(name="gmask", bufs=2) as gmp,
            tc.tile_pool(name="psS", bufs=2, space="PSUM") as psS,
            tc.tile_pool(name="psB", bufs=2, space="PSUM") as psB,
            tc.tile_pool(name="psO", bufs=2, space="PSUM") as psO,
        ):
            # ---- persistent weights (wq/wk first: they gate chunk-0 proj) ----
            wq_sb = wpool.tile([P, KO, GD], BF16, tag="wq")
            wk_sb = wpool.tile([P, KO, GD], BF16, tag="wk")
            wv_sb = wpool.tile([P, KO, GD], BF16, tag="wv")
            wo_sb = wpool.tile([P, 2, D], BF16, tag="wo")
            nc.scalar.dma_start(wq_sb, wqT[:, :, :])
            nc.sync.dma_start(wk_sb, wkT[:, :, :])

            vcol = wpool.tile([P, 1], BF16, tag="vcol")
            nc.vector.memset(vcol, 1.0)

            # PE warm-up: dummy matmuls while the first DMAs stream, so the
            # HAM clock-gate opens before the projections start
            warm = wpool.tile([P, NQ], BF16, tag="warm")
            nc.vector.memset(warm, 1.0)
            wps = psB.tile([P, NQ], F32, tag="mm1", name="wps")
            for i in range(WARM_N):
                nc.tensor.matmul(wps, warm[:, :P], warm,
                                 start=(i == 0), stop=(i == WARM_N - 1))

            # ---- persistent projection outputs ----
            qT_sb = [perm.tile([P, S], BF16, tag=f"qT{i}", name=f"qT{i}")
                     for i in range(2)]
            kT_sb = [perm.tile([P, S], BF16, tag=f"kT{i}", name=f"kT{i}")
                     for i in range(2)]
            v_sb = [perm.tile([P, HG, DK + 1], BF16, tag=f"v{i}", name=f"v{i}")
                    for i in range(KB)]

            eps_ap = wpool.tile([1, 1], F32, tag="eps")
            nc.vector.memset(eps_ap, 1e-30)

            # ones column of every persistent v tile, written once
            for i in range(KB):
                nc.gpsimd.tensor_copy(
                    out=v_sb[i][:, :, DK:DK + 1],
                    in_=vcol[:, None, :].to_broadcast((P, HG, 1)))

            def load_chunk(c, engines=(None, None)):
                # stream x slices for sequence chunk c (q, k, v) as two
                # half-depth transfers per tensor (contiguous per partition)
                eq, ek = engines
                tiles = []
                for ti, xTr in enumerate((xqT, xkT, xvT)):
                    xs = xsp.tile([P, KO, NQ], BF16, tag="xs", name="xs")
                    for hf in range(2):
                        e = (eq if ti == 0 else ek) or nc.sync
                        e.dma_start(
                            xs[:, hf * 4:(hf + 1) * 4, :],
                            xTr[:, c, hf * 4:(hf + 1) * 4, :])
                    tiles.append(xs)
                return tiles

            def proj_qk_units(c, xst):
                units = []
                for xs, w_sb, dst in ((xst[0], wq_sb, qT_sb),
                                      (xst[1], wk_sb, kT_sb)):
                    for hp in range(2):
                        def u(xs=xs, w_sb=w_sb, dst=dst, hp=hp):
                            ps = psB.tile([P, NQ], F32, tag="mm1", name="ps_qk")
                            for ko in range(KO):
                                nc.tensor.matmul(
                                    ps[:, :],
                                    w_sb[:, ko, hp * P:(hp + 1) * P],
                                    xs[:, ko, :],
                                    start=(ko == 0), stop=(ko == KO - 1),
                                )
                            nc.vector.tensor_copy(
                                out=dst[hp][:, c * NQ:(c + 1) * NQ], in_=ps[:, :])
                        units.append(u)
                return units

            def proj_v_units(c, xst):
                units = []
                xs = xst[2]
                for si in range(4):
                    def u(si=si, xs=xs, c=c):
                        sq = 4 * c + si
                        ps = psB.tile([P, NQ], F32, tag="mm1", name="ps_v")
                        for ko in range(KO):
                            nc.tensor.matmul(
                                ps[:, :GD],
                                xs[:, ko, si * P:(si + 1) * P],
                                wv_sb[:, ko, :],
                                start=(ko == 0), stop=(ko == KO - 1),
                            )
                        nc.vector.tensor_copy(
                            out=v_sb[sq][:, :, 0:DK],
                            in_=ps[:, :GD].rearrange("p (h d) -> p h d", h=HG))
                    units.append(u)
                return units

            def outproj_units(qb, O_sb, tail=False):
                osb = outp.tile([P, KO, NQ], BF16, tag="osb", name="osb")
                # on the tail (no exp left) spread the PSUM->SBUF casts
                # across engines so the drain pipelines; mid-kernel keep
                # scalar free for exp
                cast_engs = ((nc.vector, nc.scalar) if tail
                             else (nc.vector,))
                units = []
                for od in range(KO):
                    def u(od=od):
                        po = psB.tile([P, NQ], F32, tag="mm1", name="po")
                        for t in range(2):
                            nc.tensor.matmul(
                                po[:, :],
                                wo_sb[:, t, od * P:(od + 1) * P],
                                O_sb[t],
                                start=(t == 0), stop=(t == 1),
                            )
                        eng = cast_engs[od % len(cast_engs)]
                        if eng is nc.scalar:
                            eng.activation(
                                out=osb[:, od, :], in_=po[:, :],
                                func=mybir.ActivationFunctionType.Copy)
                        else:
                            eng.tensor_copy(out=osb[:, od, :], in_=po[:, :])
                    units.append(u)

                def fin(hf):
                    nc.sync.dma_start(outT[qb, :, hf * 2:(hf + 1) * 2, :],
                                      osb[:, hf * 2:(hf + 1) * 2, :])
                # stores trail the casts so the last transfer is small
                out_units = []
                for od in range(KO):
                    out_units.append(units[od])
                    if od % 2 == 1 and od < KO - 1:
                        out_units.append(lambda hf=od // 2: fin(hf))
                out_units.append(lambda: fin(3))
                return out_units

            def attention_qb(qb, micro):
                # micro: list of pending micro-op closures to weave between
                # kb iterations (keeps the in-order PE queue stall-free)
                nkb = 4 * (qb + 1) if mode == "tril" else KB

                mgf = None
                if mode == "general":
                    mgf = gmp.tile([P, KB, NQ], BF16, tag="mgf", name="mgf")
                    nc.sync.dma_start(
                        mgf,
                        maskT.rearrange("(kb p) q -> p kb q", p=P)[
                            :, :, qb * NQ:(qb + 1) * NQ])

                O_sb = [obp.tile([P, NQ], BF16, tag=f"O{i}", name=f"O{i}")
                        for i in range(2)]
                total_iters = 2 * nkb
                it = 0
                for hp in range(2):
                    pso = [psO.tile([DK + 1, NQ], F32, tag="O", name=f"pso{hh}")
                           for hh in range(2)]

                    def issue_av(pend):
                        kb, es, q0 = pend
                        for hh in range(2):
                            nc.tensor.matmul(
                                pso[hh][:, q0:],
                                v_sb[kb][:, 2 * hp + hh, :],
                                es[:, hh, q0:],
                                start=(kb == 0), stop=(kb == nkb - 1),
                            )

                    pend = []
                    for kb in range(nkb):
                        # causal trim: diagonal-straddling blocks only need
                        # queries >= the block's first key
                        q0 = 0
                        diag = mode == "tril" and kb >= 4 * qb
                        if diag:
                            q0 = (kb - 4 * qb) * P
                        sp = psS.tile([P, 2, NQ], F32, tag="mm2", name="sp")
                        es = esp.tile([P, 2, NQ], BF16, tag="es", name="es")
                        for hh in range(2):
                            nc.tensor.matmul(
                                sp[:, hh, q0:],
                                kT_sb[hp][hh * DK:(hh + 1) * DK,
                                          kb * P:(kb + 1) * P],
                                qT_sb[hp][hh * DK:(hh + 1) * DK,
                                          qb * NQ + q0:(qb + 1) * NQ],
                                start=True, stop=True,
                            )
                        nc.scalar.activation(
                            out=es[:, :, q0:], in_=sp[:, :, q0:],
                            func=mybir.ActivationFunctionType.Exp, scale=0.125)
                        if diag:
                            nc.vector.tensor_mul(
                                out=es[:, :, q0:q0 + P],
                                in0=es[:, :, q0:q0 + P],
                                in1=maskf[:, None, :].to_broadcast((P, 2, P)))
                        elif mode == "general":
                            nc.vector.tensor_mul(
                                out=es[:], in0=es[:],
                                in1=mgf[:, kb, None, :].to_broadcast((P, 2, NQ)))
                        # software pipeline: issue AV two blocks behind --
                        # its es is long ready, so the PE streams it during
                        # this block's exp latency instead of stalling
                        pend.append((kb, es, q0))
                        if len(pend) > 1:
                            issue_av(pend.pop(0))
                        it += 1
                        # weave pending micro-ops between kb steps
                        rem = total_iters - it
                        if micro:
                            n = max(1, -(-len(micro) // max(rem, 1)))
                            for _ in range(min(n, len(micro))):
                                micro.pop(0)()
                    for p in pend:
                        issue_av(p)
                    # normalize: O = O_unnorm * (1/sum); sum row comes from
                    # the ones-column. Phase-ordered across the two heads so
                    # the DVE/gpsimd chains pipeline instead of serializing.
                    sums, recips, bcs = [], [], []
                    for hh in range(2):
                        sum_sb = smallp.tile([1, NQ], F32, tag="sum",
                                             name="sum_sb")
                        if mode == "general":
                            nc.scalar.activation(
                                out=sum_sb, in_=pso[hh][DK:DK + 1, :],
                                func=mybir.ActivationFunctionType.Identity,
                                bias=eps_ap, scale=1.0)
                        else:
                            nc.vector.tensor_copy(
                                out=sum_sb, in_=pso[hh][DK:DK + 1, :])
                        sums.append(sum_sb)
                    for hh in range(2):
                        recip_sb = smallp.tile([1, NQ], F32, tag="recip",
                                               name="recip_sb")
                        nc.vector.reciprocal_approx_fast(
                            out=recip_sb, in_=sums[hh])
                        recips.append(recip_sb)
                    for hh in range(2):
                        bc_sb = smallp.tile([DK, NQ], F32, tag="bc",
                                            name="bc_sb")
                        nc.gpsimd.partition_broadcast(bc_sb, recips[hh])
                        bcs.append(bc_sb)
                    for hh in range(2):
                        nc.vector.tensor_mul(
                            out=O_sb[hp][hh * DK:(hh + 1) * DK, :],
                            in0=pso[hh][0:DK, :], in1=bcs[hh])

                return O_sb

            # ---- prologue: chunk 0 ----
            xst = load_chunk(0, engines=(nc.scalar, nc.sync))
            nc.scalar.dma_start(wv_sb, wvT[:, :, :])
            maskf = None
            if mode == "tril":
                maskf = wpool.tile([P, P], BF16, tag="maskf")
                nc.scalar.dma_start(maskf, maskd[:, :])
            nc.sync.dma_start(wo_sb, woT[:, :, :])
            # chunk-0 projections: q/k for hp0+hp1, then v
            for u in proj_qk_units(0, xst):
                u()
            for u in proj_v_units(0, xst):
                u()

            # ---- steady state: attention(c) with proj(c+1) + outproj(c-1)
            # woven into its kb loop ----
            prev = None
            for c in range(QB):
                micro = []
                if c + 1 < QB:
                    xst = load_chunk(c + 1)
                    qk = proj_qk_units(c + 1, xst)
                    vv = proj_v_units(c + 1, xst)
                    micro += qk[:2] + vv[:2] + qk[2:] + vv[2:]
                if prev is not None:
                    micro += outproj_units(*prev)
                O_sb = attention_qb(c, micro)
                for u in micro:
                    u()
                prev = (c, O_sb)
            for u in outproj_units(*prev, tail=True):
                u()

    nc.compile()
    return nc


_CACHE = {}


def _get(mode):
    if mode not in _CACHE:
        _CACHE[mode] = build(mode)
    return _CACHE[mode]


def kernel(Q, K, V, Wq, Wk, Wv, Wo, mask, _want_results=False):
    Q = np.asarray(Q, dtype=np.float32)
    K = np.asarray(K, dtype=np.float32)
    V = np.asarray(V, dtype=np.float32)
    Wq = np.asarray(Wq, dtype=np.float32)
    Wk = np.asarray(Wk, dtype=np.float32)
    Wv = np.asarray(Wv, dtype=np.float32)
    Wo = np.asarray(Wo, dtype=np.float32)
    m2 = np.asarray(mask).reshape(S, S)

    if np.array_equal(m2, np.tril(np.ones((S, S), m2.dtype))):
        mode = "tril"
    elif np.all(m2 != 0):
        mode = "ones"
    else:
        mode = "general"

    nc = _get(mode)

    def xlayout(x):
        # [S, D] -> [P, QB, KO, NQ] with x[s, d] at [d % P, s // NQ,
        # d // P, s % NQ]: every DMA line is contiguous per partition
        return np.ascontiguousarray(
            x.T.reshape(KO, P, QB, NQ).transpose(1, 2, 0, 3)).astype(BFNP)

    def wlayout(WT):
        # [D, GD] -> [P, KO, GD]
        return np.ascontiguousarray(
            WT.reshape(KO, P, GD).transpose(1, 0, 2)).astype(BFNP)

    xT = {}
    for b in range(B):
        xT[("q", b)] = xlayout(Q[b])
        xT[("k", b)] = xlayout(K[b])
        xT[("v", b)] = xlayout(V[b])

    mT = None
    maskd = None
    if mode == "general":
        mT = np.ascontiguousarray((m2.T != 0).astype(BFNP))
    elif mode == "tril":
        # diagonal-block triangle: key p <= query q'
        maskd = np.triu(np.ones((P, P), np.float32)).astype(BFNP)

    in_maps = []
    for c in range(8):
        b, g = divmod(c, 4)
        sl = slice(g * GD, (g + 1) * GD)
        im = {
            "xqT": xT[("q", b)],
            "xkT": xT[("k", b)],
            "xvT": xT[("v", b)],
            "wqT": wlayout(Wq[sl, :].T),
            "wkT": wlayout(Wk[sl, :].T),
            "wvT": wlayout(Wv[sl, :].T),
            "woT": np.ascontiguousarray(
                Wo[:, sl].T.reshape(2, P, D).transpose(1, 0, 2)).astype(BFNP),
        }
        if mode == "tril":
            im["maskd"] = maskd
        elif mode == "general":
            im["maskT"] = mT
        in_maps.append(im)

    res = run_bass_kernel_spmd(nc, in_maps, core_ids=list(range(8)))

    out = np.empty((B, S, D), dtype=np.float32)
    for b in range(B):
        acc = res.results[4 * b]["outT"].astype(np.float32)
        for g in range(1, 4):
            acc += res.results[4 * b + g]["outT"].astype(np.float32)
        # [QB, P, KO, NQ] -> [S, D]
        out[b] = acc.transpose(0, 3, 2, 1).reshape(S, D)
    if _want_results:
        return out, res
    return out


# revision 47
# speedup vs baseline: 1.0479x; 1.0479x over previous
"""Multi-head causal attention (B=2, S=2048, D=1024, H=16) on 8 trn2 cores.

Sharding: core c -> batch b = c//4, head-group g = c%4 (4 heads each).
Tensor-parallel on Wq/Wk/Wv (column) and Wo (row); the Wo all-reduce is the
host-side sum of the 4 per-core partials of each batch.

v2 changes over the fp32 baseline:
 - bf16 activations/weights end-to-end (host casts); PSUM stays fp32.
   Halves HBM traffic (37.6 -> ~19 MB/core) and SBUF footprint.
 - Causal diagonal trim: for diagonal-straddling key blocks only the
   query range >= the block's first key is scored/exp'd/accumulated.
   Cuts ACT exp work ~15% and trims score/AV streams.
 - Softmax sum reciprocal straight from PSUM on DVE (drops the ACT
   identity+eps hop; causal rows always have >= 1 unmasked key).
 - Batched DMA (2 issues per tensor-chunk, 1 output DMA per query
   superblock); chunk loads ride the sync queue so exp never queues
   behind DMA-issue on the scalar engine.
 - Fine-grained issue interleave: projection / output-projection
   micro-ops are woven between attention kb-steps so the in-order PE
   queue always has independent work adjacent (no head-of-line stall
   when exp rate-limits the attention inner loop).

Device layout notes (unchanged):
 - Activations stay transposed (features on partitions): every matmul
   contraction is on partitions with zero on-chip transposes.
 - Scores are built as S.T (keys on partitions, queries free); softmax
   sums come free via an appended ones-column of V.
 - No row-max subtraction: scores ~N(0, 0.4) after the 1/8 scale.
 - Causal mask applied post-exp, only on the [128,128] triangle of
   diagonal blocks.
"""

import numpy as np
import ml_dtypes

import concourse.bacc as bacc
import concourse.mybir as mybir
import concourse.tile as tile
from concourse.bass_utils import run_bass_kernel_spmd

B, S, D, H = 2, 2048, 1024, 16
DK = 64            # head dim
HG = 4             # heads per core
GD = HG * DK       # 256 dims per head-group
P = 128
NQ = 512           # query chunk (free dim of score blocks)
QB = S // NQ       # 4 query superblocks
KB = S // P        # 16 key blocks
KO = D // P        # 8 contraction tiles for the projections
F32 = mybir.dt.float32
BF16 = mybir.dt.bfloat16
BFNP = ml_dtypes.bfloat16
WARM_N = 14


def build(mode):
    assert mode in ("tril", "ones", "general")
    nc = bacc.Bacc(None, target_bir_lowering=False)

    # All DRAM layouts are partition-contiguous (host pre-arranges): every
    # DMA is 128 lines of >= 4KB, so HWDGE descriptor generation is cheap.
    xqT = nc.dram_tensor("xqT", [P, QB, KO, NQ], BF16, kind="ExternalInput")
    xkT = nc.dram_tensor("xkT", [P, QB, KO, NQ], BF16, kind="ExternalInput")
    xvT = nc.dram_tensor("xvT", [P, QB, KO, NQ], BF16, kind="ExternalInput")
    wqT = nc.dram_tensor("wqT", [P, KO, GD], BF16, kind="ExternalInput")
    wkT = nc.dram_tensor("wkT", [P, KO, GD], BF16, kind="ExternalInput")
    wvT = nc.dram_tensor("wvT", [P, KO, GD], BF16, kind="ExternalInput")
    woT = nc.dram_tensor("woT", [P, 2, D], BF16, kind="ExternalInput")
    maskd = maskT = None
    if mode == "tril":
        # [P, P] upper-triangular (key p <= query q') bf16 pattern
        maskd = nc.dram_tensor("maskd", [P, P], BF16, kind="ExternalInput")
    elif mode == "general":
        maskT = nc.dram_tensor("maskT", [S, S], BF16, kind="ExternalInput")
    outT = nc.dram_tensor("outT", [QB, P, KO, NQ], BF16, kind="ExternalOutput")

    with tile.TileContext(nc) as tc:
        with (
            tc.tile_pool(name="wpool", bufs=1) as wpool,
            tc.tile_pool(name="perm", bufs=1) as perm,
            tc.tile_pool(name="xs", bufs=6) as xsp,
            tc.tile_pool(name="es", bufs=6) as esp,
            tc.tile_pool(name="ob", bufs=2) as obp,
            tc.tile_pool(name="outp", bufs=2) as outp,
            tc.tile_pool(name="small", bufs=4) as smallp,
            tc.tile_pool(name="gmask", bufs=2) as gmp,
            tc.tile_pool(name="psS", bufs=2, space="PSUM") as psS,
            tc.tile_pool(name="psB", bufs=2, space="PSUM") as psB,
            tc.tile_pool(name="psO", bufs=2, space="PSUM") as psO,
        ):
            # ---- persistent weights (wq/wk first: they gate chunk-0 proj) ----
            wq_sb = wpool.tile([P, KO, GD], BF16, tag="wq")
            wk_sb = wpool.tile([P, KO, GD], BF16, tag="wk")
            wv_sb = wpool.tile([P, KO, GD], BF16, tag="wv")
            wo_sb = wpool.tile([P, 2, D], BF16, tag="wo")
            nc.scalar.dma_start(wq_sb, wqT[:, :, :])

            vcol = wpool.tile([P, 1], BF16, tag="vcol")
            nc.vector.memset(vcol, 1.0)

            # PE warm-up: dummy matmuls while the first DMAs stream, so the
            # HAM clock-gate opens before the projections start
            # short N=128 warm matmuls: enough PE activity to open the HAM
            # clock gate without queue-delaying the first projection once
            # the weight/activation DMAs land
            warm = wpool.tile([P, NQ], BF16, tag="warm")
            nc.vector.memset(warm, 1.0)
            wps = psB.tile([P, NQ], F32, tag="mm1", name="wps")
            for i in range(WARM_N):
                nc.tensor.matmul(wps[:, :P], warm[:, :P], warm[:, :P],
                                 start=(i == 0), stop=(i == WARM_N - 1))

            # ---- persistent projection outputs ----
            qT_sb = [perm.tile([P, S], BF16, tag=f"qT{i}", name=f"qT{i}")
                     for i in range(2)]
            kT_sb = [perm.tile([P, S], BF16, tag=f"kT{i}", name=f"kT{i}")
                     for i in range(2)]
            v_sb = [perm.tile([P, HG, DK + 1], BF16, tag=f"v{i}", name=f"v{i}")
                    for i in range(KB)]

            eps_ap = wpool.tile([1, 1], F32, tag="eps")
            nc.vector.memset(eps_ap, 1e-30)

            # ones column of every persistent v tile, written once
            for i in range(KB):
                nc.gpsimd.tensor_copy(
                    out=v_sb[i][:, :, DK:DK + 1],
                    in_=vcol[:, None, :].to_broadcast((P, HG, 1)))

            def load_chunk(c, engines=(None, None)):
                # stream x slices for sequence chunk c (q, k, v) as two
                # half-depth transfers per tensor (contiguous per partition)
                eq, ek = engines
                tiles = []
                for ti, xTr in enumerate((xqT, xkT, xvT)):
                    xs = xsp.tile([P, KO, NQ], BF16, tag="xs", name="xs")
                    for hf in range(2):
                        e = (eq if ti == 0 else ek) or nc.sync
                        e.dma_start(
                            xs[:, hf * 4:(hf + 1) * 4, :],
                            xTr[:, c, hf * 4:(hf + 1) * 4, :])
                    tiles.append(xs)
                return tiles

            def proj_qk_units(c, xst):
                units = []
                for xs, w_sb, dst in ((xst[0], wq_sb, qT_sb),
                                      (xst[1], wk_sb, kT_sb)):
                    for hp in range(2):
                        def u(xs=xs, w_sb=w_sb, dst=dst, hp=hp):
                            ps = psB.tile([P, NQ], F32, tag="mm1", name="ps_qk")
                            for ko in range(KO):
                                nc.tensor.matmul(
                                    ps[:, :],
                                    w_sb[:, ko, hp * P:(hp + 1) * P],
                                    xs[:, ko, :],
                                    start=(ko == 0), stop=(ko == KO - 1),
                                )
                            nc.vector.tensor_copy(
                                out=dst[hp][:, c * NQ:(c + 1) * NQ], in_=ps[:, :])
                        units.append(u)
                return units

            def proj_v_units(c, xst):
                units = []
                xs = xst[2]
                for si in range(4):
                    def u(si=si, xs=xs, c=c):
                        sq = 4 * c + si
                        ps = psB.tile([P, NQ], F32, tag="mm1", name="ps_v")
                        for ko in range(KO):
                            nc.tensor.matmul(
                                ps[:, :GD],
                                xs[:, ko, si * P:(si + 1) * P],
                                wv_sb[:, ko, :],
                                start=(ko == 0), stop=(ko == KO - 1),
                            )
                        nc.vector.tensor_copy(
                            out=v_sb[sq][:, :, 0:DK],
                            in_=ps[:, :GD].rearrange("p (h d) -> p h d", h=HG))
                    units.append(u)
                return units

            def outproj_units(qb, O_sb, tail=False):
                osb = outp.tile([P, KO, NQ], BF16, tag="osb", name="osb")
                # on the tail (no exp left) spread the PSUM->SBUF casts
                # across engines so the drain pipelines; mid-kernel keep
                # scalar free for exp
                cast_engs = ((nc.vector, nc.scalar) if tail
                             else (nc.vector,))
                units = []
                for od in range(KO):
                    def u(od=od):
                        po = psB.tile([P, NQ], F32, tag="mm1", name="po")
                        for t in range(2):
                            nc.tensor.matmul(
                                po[:, :],
                                wo_sb[:, t, od * P:(od + 1) * P],
                                O_sb[t],
                                start=(t == 0), stop=(t == 1),
                            )
                        eng = cast_engs[od % len(cast_engs)]
                        if eng is nc.scalar:
                            eng.activation(
                                out=osb[:, od, :], in_=po[:, :],
                                func=mybir.ActivationFunctionType.Copy)
                        else:
                            eng.tensor_copy(out=osb[:, od, :], in_=po[:, :])
                    units.append(u)

                def fin(hf):
                    nc.sync.dma_start(outT[qb, :, hf * 2:(hf + 1) * 2, :],
                                      osb[:, hf * 2:(hf + 1) * 2, :])
                # stores trail the casts so the last transfer is small
                out_units = []
                for od in range(KO):
                    out_units.append(units[od])
                    if od % 2 == 1 and od < KO - 1:
                        out_units.append(lambda hf=od // 2: fin(hf))
                out_units.append(lambda: fin(3))
                return out_units

            def attention_qb(qb, micro):
                # micro: list of pending micro-op closures to weave between
                # kb iterations (keeps the in-order PE queue stall-free)
                nkb = 4 * (qb + 1) if mode == "tril" else KB

                mgf = None
                if mode == "general":
                    mgf = gmp.tile([P, KB, NQ], BF16, tag="mgf", name="mgf")
                    nc.sync.dma_start(
                        mgf,
                        maskT.rearrange("(kb p) q -> p kb q", p=P)[
                            :, :, qb * NQ:(qb + 1) * NQ])

                O_sb = [obp.tile([P, NQ], BF16, tag=f"O{i}", name=f"O{i}")
                        for i in range(2)]
                total_iters = 2 * nkb
                it = 0
                for hp in range(2):
                    pso = [psO.tile([DK + 1, NQ], F32, tag="O", name=f"pso{hh}")
                           for hh in range(2)]

                    def issue_av(pend):
                        kb, es, q0 = pend
                        for hh in range(2):
                            nc.tensor.matmul(
                                pso[hh][:, q0:],
                                v_sb[kb][:, 2 * hp + hh, :],
                                es[:, hh, q0:],
                                start=(kb == 0), stop=(kb == nkb - 1),
                            )

                    pend = []
                    for kb in range(nkb):
                        # causal trim: diagonal-straddling blocks only need
                        # queries >= the block's first key
                        q0 = 0
                        diag = mode == "tril" and kb >= 4 * qb
                        if diag:
                            q0 = (kb - 4 * qb) * P
                        sp = psS.tile([P, 2, NQ], F32, tag="mm2", name="sp")
                        es = esp.tile([P, 2, NQ], BF16, tag="es", name="es")
                        for hh in range(2):
                            nc.tensor.matmul(
                                sp[:, hh, q0:],
                                kT_sb[hp][hh * DK:(hh + 1) * DK,
                                          kb * P:(kb + 1) * P],
                                qT_sb[hp][hh * DK:(hh + 1) * DK,
                                          qb * NQ + q0:(qb + 1) * NQ],
                                start=True, stop=True,
                            )
                        nc.scalar.activation(
                            out=es[:, :, q0:], in_=sp[:, :, q0:],
                            func=mybir.ActivationFunctionType.Exp, scale=0.125)
                        if diag:
                            nc.vector.tensor_mul(
                                out=es[:, :, q0:q0 + P],
                                in0=es[:, :, q0:q0 + P],
                                in1=maskf[:, None, :].to_broadcast((P, 2, P)))
                        elif mode == "general":
                            nc.vector.tensor_mul(
                                out=es[:], in0=es[:],
                                in1=mgf[:, kb, None, :].to_broadcast((P, 2, NQ)))
                        # software pipeline: issue AV two blocks behind --
                        # its es is long ready, so the PE streams it during
                        # this block's exp latency instead of stalling
                        pend.append((kb, es, q0))
                        if len(pend) > 1:
                            issue_av(pend.pop(0))
                        it += 1
                        # weave pending micro-ops between kb steps
                        rem = total_iters - it
                        if micro:
                            n = max(1, -(-len(micro) // max(rem, 1)))
                            for _ in range(min(n, len(micro))):
                                micro.pop(0)()
                    for p in pend:
                        issue_av(p)
                    # normalize: O = O_unnorm * (1/sum); sum row comes from
                    # the ones-column. Phase-ordered across the two heads so
                    # the DVE/gpsimd chains pipeline instead of serializing.
                    sums, recips, bcs = [], [], []
                    for hh in range(2):
                        sum_sb = smallp.tile([1, NQ], F32, tag="sum",
                                             name="sum_sb")
                        if mode == "general":
                            nc.scalar.activation(
                                out=sum_sb, in_=pso[hh][DK:DK + 1, :],
                                func=mybir.ActivationFunctionType.Identity,
                                bias=eps_ap, scale=1.0)
                        else:
                            nc.vector.tensor_copy(
                                out=sum_sb, in_=pso[hh][DK:DK + 1, :])
                        sums.append(sum_sb)
                    for hh in range(2):
                        recip_sb = smallp.tile([1, NQ], F32, tag="recip",
                                               name="recip_sb")
                        nc.vector.reciprocal_approx_fast(
                            out=recip_sb, in_=sums[hh])
                        recips.append(recip_sb)
                    for hh in range(2):
                        bc_sb = smallp.tile([DK, NQ], F32, tag="bc",
                                            name="bc_sb")
                        nc.gpsimd.partition_broadcast(bc_sb, recips[hh])
                        bcs.append(bc_sb)
                    for hh in range(2):
                        nc.vector.tensor_mul(
                            out=O_sb[hp][hh * DK:(hh + 1) * DK, :],
                            in0=pso[hh][0:DK, :], in1=bcs[hh])

                return O_sb

            # ---- prologue: chunk 0 ----
            # chunk-0 activations all on sync (q first), weights on scalar:
            # the two queues issue + transfer in parallel, so wq and xq both
            # land earlier and the first projection starts sooner
            xst = load_chunk(0, engines=(nc.sync, nc.sync))
            nc.scalar.dma_start(wk_sb, wkT[:, :, :])
            nc.scalar.dma_start(wv_sb, wvT[:, :, :])
            maskf = None
            if mode == "tril":
                maskf = wpool.tile([P, P], BF16, tag="maskf")
                nc.scalar.dma_start(maskf, maskd[:, :])
            nc.scalar.dma_start(wo_sb, woT[:, :, :])
            # chunk-0 projections: q/k for hp0+hp1, then v
            for u in proj_qk_units(0, xst):
                u()
            for u in proj_v_units(0, xst):
                u()

            # ---- steady state: attention(c) with proj(c+1) + outproj(c-1)
            # woven into its kb loop ----
            prev = None
            for c in range(QB):
                micro = []
                hold = None
                if c + 1 < QB:
                    xst = load_chunk(c + 1)
                    qk = proj_qk_units(c + 1, xst)
                    vv = proj_v_units(c + 1, xst)
                    micro += qk[:2] + vv[:2] + qk[2:] + vv[2:]
                if prev is not None:
                    ou = outproj_units(*prev)
                    if c == QB - 1:
                        # hold outproj(QB-2) back from the weave: the last
                        # attention phase is PE-bound anyway, and issuing it
                        # here keeps the PE busy during the final qb's
                        # normalize chain latency
                        hold = ou
                    else:
                        micro += ou
                O_sb = attention_qb(c, micro)
                for u in micro:
                    u()
                if hold:
                    for u in hold:
                        u()
                prev = (c, O_sb)
            for u in outproj_units(*prev, tail=True):
                u()

    nc.compile()
    return nc


_CACHE = {}


def _get(mode):
    if mode not in _CACHE:
        _CACHE[mode] = build(mode)
    return _CACHE[mode]


def kernel(Q, K, V, Wq, Wk, Wv, Wo, mask, _want_results=False):
    Q = np.asarray(Q, dtype=np.float32)
    K = np.asarray(K, dtype=np.float32)
    V = np.asarray(V, dtype=np.float32)
    Wq = np.asarray(Wq, dtype=np.float32)
    Wk = np.asarray(Wk, dtype=np.float32)
    Wv = np.asarray(Wv, dtype=np.float32)
    Wo = np.asarray(Wo, dtype=np.float32)
    m2 = np.asarray(mask).reshape(S, S)

    if np.array_equal(m2, np.tril(np.ones((S, S), m2.dtype))):
        mode = "tril"
    elif np.all(m2 != 0):
        mode = "ones"
    else:
        mode = "general"

    nc = _get(mode)

    def xlayout(x):
        # [S, D] -> [P, QB, KO, NQ] with x[s, d] at [d % P, s // NQ,
        # d // P, s % NQ]: every DMA line is contiguous per partition
        return np.ascontiguousarray(
            x.T.reshape(KO, P, QB, NQ).transpose(1, 2, 0, 3)).astype(BFNP)

    def wlayout(WT):
        # [D, GD] -> [P, KO, GD]
        return np.ascontiguousarray(
            WT.reshape(KO, P, GD).transpose(1, 0, 2)).astype(BFNP)

    xT = {}
    for b in range(B):
        xT[("q", b)] = xlayout(Q[b])
        xT[("k", b)] = xlayout(K[b])
        xT[("v", b)] = xlayout(V[b])

    mT = None
    maskd = None
    if mode == "general":
        mT = np.ascontiguousarray((m2.T != 0).astype(BFNP))
    elif mode == "tril":
        # diagonal-block triangle: key p <= query q'
        maskd = np.triu(np.ones((P, P), np.float32)).astype(BFNP)

    in_maps = []
    for c in range(8):
        b, g = divmod(c, 4)
        sl = slice(g * GD, (g + 1) * GD)
        im = {
            "xqT": xT[("q", b)],
            "xkT": xT[("k", b)],
            "xvT": xT[("v", b)],
            "wqT": wlayout(Wq[sl, :].T),
            "wkT": wlayout(Wk[sl, :].T),
            "wvT": wlayout(Wv[sl, :].T),
            "woT": np.ascontiguousarray(
                Wo[:, sl].T.reshape(2, P, D).transpose(1, 0, 2)).astype(BFNP),
        }
        if mode == "tril":
            im["maskd"] = maskd
        elif mode == "general":
            im["maskT"] = mT
        in_maps.append(im)

    res = run_bass_kernel_spmd(nc, in_maps, core_ids=list(range(8)))

    out = np.empty((B, S, D), dtype=np.float32)
    for b in range(B):
        acc = res.results[4 * b]["outT"].astype(np.float32)
        for g in range(1, 4):
            acc += res.results[4 * b + g]["outT"].astype(np.float32)
        # [QB, P, KO, NQ] -> [S, D]
        out[b] = acc.transpose(0, 3, 2, 1).reshape(S, D)
    if _want_results:
        return out, res
    return out
